# revision 3
# baseline (speedup 1.0000x reference)
"""Trainium2 Bass kernel for nn_Attention_49598282334528 (v2).

Dense transformer attention block: fused QKV projection + RoPE + causal
GQA attention + output projection, for
  x: [2, 2048, 2048], H=16 q heads, KV=4 kv heads, head_dim=128.

Sharding (8 NeuronCores): data-parallel over batch (2) x tensor-parallel
over kv-head groups (4).  Core c handles batch c//4, kv-group c%4 (4 q
heads + 1 kv head).  Each core computes a full-width partial of the
output projection (row-parallel Wo); the host sums the 4 partials per
batch and stacks batches.

v2 layout/scheduling (vs v1):
  - all PE operands bf16 (PSUM accumulation stays f32): removes the
    fp32r sub-256-free-dim penalty, halves DMA bytes and SBUF footprint.
    RoPE pair-interleave is pre-permuted into Wq/Wk columns host-side.
  - single software-pipelined loop over 4 seq-blocks of 512:
    proj rounds -> attention -> out-projection per block, so the
    phase-transition PE bubbles of v1 disappear.
  - QKV projection runs in 3 rounds of 2 outputs ({k,q0},{q1,q2},{q3,v})
    so RoPE of round i (DVE) hides under round i+1's matmuls and PSUM
    stays within 8 banks.
  - v is projected directly into [s, e] orientation (lhsT = x chunk),
    eliminating v PE-transposes.
  - causal diagonal handled by multiplying exp scores with a 0/1
    triangular tile on DVE (no PE mask matmuls).
  - startup DMAs chunked so the first matmul starts at ~2us.
"""

import sys

if "/opt/trn_rl_repo" not in sys.path:
    sys.path.insert(0, "/opt/trn_rl_repo")

import numpy as np

B, S, D = 2, 2048, 2048
H, KV, HD = 16, 4, 128
G = 4                # kv groups == cores per batch
QPH = H // KV        # q heads per group = 4
EQ = QPH * HD        # per-core q width = 512
NCORES = 8
P = 128
ABLK = 512           # seq block
NA = S // ABLK       # 4
ND = D // P          # 16 contraction chunks
SCALE = 1.0 / float(np.sqrt(HD))

_CACHE = {}


def _build_program():
    import concourse.bass as bass
    import concourse.tile as tile
    from concourse import bacc, mybir

    f32 = mybir.dt.float32
    bf16 = mybir.dt.bfloat16
    EXP = mybir.ActivationFunctionType.Exp
    COPY = mybir.ActivationFunctionType.Copy

    nc = bacc.Bacc("TRN2", target_bir_lowering=False, debug=False)

    xt = nc.dram_tensor("xt", [D, S], bf16, kind="ExternalInput").ap()
    w1 = nc.dram_tensor("w1", [D, 2 * P], bf16, kind="ExternalInput").ap()
    w2 = nc.dram_tensor("w2", [D, 2 * P], bf16, kind="ExternalInput").ap()
    w3 = nc.dram_tensor("w3", [D, 2 * P], bf16, kind="ExternalInput").ap()
    wo = nc.dram_tensor("wo", [EQ, D], bf16, kind="ExternalInput").ap()
    cosT = nc.dram_tensor("cosT", [HD // 2, S], bf16, kind="ExternalInput").ap()
    sinT = nc.dram_tensor("sinT", [HD // 2, S], bf16, kind="ExternalInput").ap()
    ones_d = nc.dram_tensor("ones_d", [P, P], bf16, kind="ExternalInput").ap()
    mask01_d = nc.dram_tensor("mask01_d", [P, P], bf16, kind="ExternalInput").ap()
    outp = nc.dram_tensor("outp", [S, D], f32, kind="ExternalOutput").ap()

    xt_r = xt.rearrange("(o p) s -> p o s", p=P)     # [128, 16, 2048]
    w1_r = w1.rearrange("(o p) e -> p o e", p=P)     # [128, 16, 256]
    w2_r = w2.rearrange("(o p) e -> p o e", p=P)
    w3_r = w3.rearrange("(o p) e -> p o e", p=P)
    wo_r = wo.rearrange("(h p) d -> p h d", p=P)     # [128, 4, 2048]

    HH = HD // 2

    with tile.TileContext(nc) as tc:
        import contextlib

        with contextlib.ExitStack() as stack:
            const = stack.enter_context(tc.tile_pool(name="const", bufs=1))
            wpool = stack.enter_context(tc.tile_pool(name="wpool", bufs=1))
            xpool = stack.enter_context(tc.tile_pool(name="xpool", bufs=1))
            qkv = stack.enter_context(tc.tile_pool(name="qkv", bufs=1))
            oTp = stack.enter_context(tc.tile_pool(name="oTp", bufs=2))
            ropet = stack.enter_context(tc.tile_pool(name="ropet", bufs=4))
            stsb = stack.enter_context(tc.tile_pool(name="stsb", bufs=6))
            rcp = stack.enter_context(tc.tile_pool(name="rcp", bufs=2))
            osb = stack.enter_context(tc.tile_pool(name="osb", bufs=4))
            # PSUM budget (8 banks): proj rounds 2 + score-pair/outproj
            # 2-bank tiles x2 + attention-out/denominator 2 (parity-swapped
            # so h-transitions wait on the cheap reciprocal, not the mul).
            projps = stack.enter_context(
                tc.tile_pool(name="projps", bufs=2, space="PSUM"))
            stps = stack.enter_context(
                tc.tile_pool(name="stps", bufs=2, space="PSUM"))
            accps = stack.enter_context(
                tc.tile_pool(name="accps", bufs=2, space="PSUM"))

            # ---- SBUF persistents ----
            cos_sb = const.tile([HH, S], bf16)
            sin_sb = const.tile([HH, S], bf16)
            ones_sb = const.tile([P, P], bf16)
            mask01_sb = const.tile([P, P], bf16)

            w1_sb = wpool.tile([P, ND, 2 * P], bf16)
            w2_sb = wpool.tile([P, ND, 2 * P], bf16)
            w3_sb = wpool.tile([P, ND, 2 * P], bf16)
            wo_sb = wpool.tile([P, QPH, D], bf16)

            xb_sb = [xpool.tile([P, ND, ABLK], bf16, name=f"xb{b}")
                     for b in range(NA)]

            qT = [qkv.tile([P, QPH, ABLK], bf16, name=f"qT{b}")
                  for b in range(NA)]
            kT = [qkv.tile([P, ABLK], bf16, name=f"kT{b}") for b in range(NA)]
            vS = [qkv.tile([P, ABLK // P, HD], bf16, name=f"v{b}")
                  for b in range(NA)]

            # ---- DMA schedule: startup chunked so PE starts ASAP.
            # Interleave w1 / x-block-0 chunks, then consts (needed by the
            # first rope), then w2/w3 just in time for their rounds.
            NCH = 4
            DCH = ND // NCH  # 4 di per chunk
            def wchunk(sb, r, c):
                nc.sync.dma_start(out=sb[:, c * DCH:(c + 1) * DCH, :],
                                  in_=r[:, c * DCH:(c + 1) * DCH, :])
            def xchunk(b, c):
                nc.sync.dma_start(
                    out=xb_sb[b][:, c * DCH:(c + 1) * DCH, :],
                    in_=xt_r[:, c * DCH:(c + 1) * DCH,
                             b * ABLK:(b + 1) * ABLK])
            wchunk(w1_sb, w1_r, 0)
            xchunk(0, 0)
            wchunk(w1_sb, w1_r, 1)
            xchunk(0, 1)
            nc.sync.dma_start(out=cos_sb[:], in_=cosT[:])
            nc.sync.dma_start(out=sin_sb[:], in_=sinT[:])
            wchunk(w1_sb, w1_r, 2)
            xchunk(0, 2)
            wchunk(w1_sb, w1_r, 3)
            xchunk(0, 3)
            wchunk(w2_sb, w2_r, 0)
            wchunk(w2_sb, w2_r, 1)
            wchunk(w2_sb, w2_r, 2)
            wchunk(w2_sb, w2_r, 3)
            nc.sync.dma_start(out=ones_sb[:], in_=ones_d[:])
            nc.sync.dma_start(out=mask01_sb[:], in_=mask01_d[:])
            nc.sync.dma_start(out=w3_sb[:], in_=w3_r[:])
            nc.sync.dma_start(out=xb_sb[1][:], in_=xt_r[:, :, ABLK:2 * ABLK])
            nc.sync.dma_start(out=wo_sb[:], in_=wo_r[:])
            nc.sync.dma_start(out=xb_sb[2][:], in_=xt_r[:, :, 2 * ABLK:3 * ABLK])
            nc.sync.dma_start(out=xb_sb[3][:], in_=xt_r[:, :, 3 * ABLK:4 * ABLK])

            def rope(src, dst, s0):
                """src: PSUM [128, ABLK] f32 (rows 0-63 even dims, 64-127
                odd dims); dst: SBUF bf16 [128, ABLK] slice."""
                ct = cos_sb[:, s0:s0 + ABLK]
                st_ = sin_sb[:, s0:s0 + ABLK]
                top = src[0:HH, :]
                bot = src[HH:P, :]
                t1 = ropet.tile([HH, ABLK], bf16, tag="t1", name="rt1")
                t2 = ropet.tile([HH, ABLK], bf16, tag="t2", name="rt2")
                nc.vector.tensor_mul(t1[:], top, ct)
                nc.vector.tensor_mul(t2[:], bot, st_)
                nc.vector.tensor_sub(dst[0:HH, :], t1[:], t2[:])
                t3 = ropet.tile([HH, ABLK], bf16, tag="t1", name="rt1")
                t4 = ropet.tile([HH, ABLK], bf16, tag="t2", name="rt2")
                nc.vector.tensor_mul(t3[:], top, st_)
                nc.vector.tensor_mul(t4[:], bot, ct)
                nc.vector.tensor_add(dst[HH:P, :], t3[:], t4[:])

            for b in range(NA):
                s0 = b * ABLK
                xb = xb_sb[b]

                # ---- projection: 6 single-output rounds (k,q0..q3,v) so
                # RoPE of round i (DVE) hides under round i+1's matmuls
                # while only 2 PSUM banks rotate.
                rounds = [
                    (w1_sb, 0, kT[b][:]),
                    (w1_sb, 1, qT[b][:, 0, :]),
                    (w2_sb, 0, qT[b][:, 1, :]),
                    (w2_sb, 1, qT[b][:, 2, :]),
                    (w3_sb, 0, qT[b][:, 3, :]),
                ]
                for wsb, col, dst in rounds:
                    rp = projps.tile([P, ABLK], f32, tag="proj", name="rp")
                    for di in range(ND):
                        nc.tensor.matmul(
                            rp[:], wsb[:, di, col * P:(col + 1) * P],
                            xb[:, di, :], start=di == 0, stop=di == ND - 1)
                    rope(rp[:], dst, s0)
                # v directly in [s, e] orientation (lhsT = x chunk)
                # accumulation groups must be sequential within a PSUM bank:
                # j outer (one group per s-tile), di inner.
                vt = projps.tile([P, ABLK // P, HD], f32, tag="proj",
                                 name="vt")
                for j in range(ABLK // P):
                    for di in range(ND):
                        nc.tensor.matmul(
                            vt[:, j, :], xb[:, di, j * P:(j + 1) * P],
                            w3_sb[:, di, P:2 * P],
                            start=di == 0, stop=di == ND - 1)
                nc.scalar.activation(vS[b][:], vt[:], COPY)

                # ---- attention for block-row b ----
                # Score tiles processed in PAIRS sharing a 2-bank PSUM
                # tile: full pairs get ONE exp over both halves; the
                # denominators of each 4 full tiles are pre-summed on DVE
                # (quad) so one ones-matmul covers them.  Pipelined one
                # pair ahead.  ot/sm slots parity-swap each head so the
                # next head's PV waits only on the reciprocal.
                n_sk = (s0 + ABLK) // P
                n_pair = n_sk // 2
                n_full = 2 * b   # full pairs per head (then 2 diag pairs)
                oT_t = oTp.tile([P, QPH, ABLK], bf16, tag="oT", name="oT")

                def kslice(ki):
                    return kT[ki // (ABLK // P)][
                        :, (ki % (ABLK // P)) * P:(ki % (ABLK // P) + 1) * P]

                def issue_pair(h, p):
                    ki0 = 2 * p
                    stp = stps.tile([P, 2, ABLK], f32, tag="st", name="stp")
                    stt = stsb.tile([P, 2, ABLK], bf16, tag="stsb",
                                    name="stt")
                    for half in range(2):
                        ki = ki0 + half
                        lead = max(ki * P - s0, 0)
                        nc.tensor.matmul(
                            stp[:, half, lead:], kslice(ki),
                            qT[b][:, h, lead:], start=True, stop=True)
                    if p < n_full:
                        nc.scalar.activation(stt[:, :, :], stp[:, :, :],
                                             EXP, scale=SCALE)
                    else:
                        for half in range(2):
                            ki = ki0 + half
                            lead = ki * P - s0
                            nc.scalar.activation(
                                stt[:, half, lead:], stp[:, half, lead:],
                                EXP, scale=SCALE)
                            nc.vector.tensor_mul(
                                stt[:, half, lead:lead + P],
                                stt[:, half, lead:lead + P], mask01_sb[:])
                    return stt

                iters = [(h, p) for h in range(QPH) for p in range(n_pair)]
                pending = issue_pair(*iters[0])
                ot = sm = qsum = None
                for idx, (h, p) in enumerate(iters):
                    stt = pending
                    if idx + 1 < len(iters):
                        pending = issue_pair(*iters[idx + 1])
                    if p == 0:
                        if h % 2 == 0:
                            ot = accps.tile([P, ABLK], f32, tag="acc",
                                            name="ot")
                            sm = accps.tile([P, ABLK], f32, tag="acc",
                                            name="sm")
                        else:
                            sm = accps.tile([P, ABLK], f32, tag="acc",
                                            name="sm")
                            ot = accps.tile([P, ABLK], f32, tag="acc",
                                            name="ot")
                    for half in range(2):
                        ki = 2 * p + half
                        lead = max(ki * P - s0, 0)
                        nc.tensor.matmul(
                            ot[:, lead:],
                            vS[ki // (ABLK // P)][:, ki % (ABLK // P), :],
                            stt[:, half, lead:],
                            start=ki == 0, stop=ki == n_sk - 1)
                    if p < n_full:
                        # denominator: accumulate 2 pairs (4 tiles) on DVE,
                        # then a single ones-matmul per quad.
                        if p % 2 == 0:
                            qsum = stsb.tile([P, ABLK], bf16, tag="qsum",
                                             name="qsum")
                            nc.vector.tensor_add(qsum[:], stt[:, 0, :],
                                                 stt[:, 1, :])
                        else:
                            nc.vector.tensor_add(qsum[:], qsum[:],
                                                 stt[:, 0, :])
                            nc.vector.tensor_add(qsum[:], qsum[:],
                                                 stt[:, 1, :])
                            nc.tensor.matmul(
                                sm[:], ones_sb[:], qsum[:],
                                start=p == 1, stop=False)
                    elif p == n_full:
                        # first diag pair: start region-wise accumulation
                        # of the 4 diagonal tiles on DVE.
                        qsum = stsb.tile([P, ABLK], bf16, tag="qsum",
                                         name="qsum")
                        nc.vector.tensor_copy(qsum[:, 0:P], stt[:, 0, 0:P])
                        nc.vector.tensor_add(qsum[:, P:], stt[:, 0, P:],
                                             stt[:, 1, P:])
                    else:
                        # second diag pair: finish the sum, single
                        # ones-matmul for all four diagonal tiles.
                        nc.vector.tensor_add(qsum[:, 2 * P:], qsum[:, 2 * P:],
                                             stt[:, 0, 2 * P:])
                        nc.vector.tensor_add(qsum[:, 3 * P:], qsum[:, 3 * P:],
                                             stt[:, 1, 3 * P:])
                        nc.tensor.matmul(
                            sm[:], ones_sb[:], qsum[:],
                            start=b == 0, stop=True)
                    if p == n_pair - 1:
                        rc = rcp.tile([P, ABLK], f32, tag="rc", name="rc")
                        nc.vector.reciprocal(rc[:], sm[:])
                        nc.vector.tensor_mul(oT_t[:, h, :], ot[:], rc[:])

                # ---- output projection for block-row b (partial) ----
                # shares the score-pair PSUM slots; two column-chunks per
                # 2-bank tile.
                op_pair = None
                for off in range(ABLK // P):
                    t = b * (ABLK // P) + off
                    for cb in range(D // ABLK):
                        if cb < 2:
                            if cb == 0:
                                op_pair = stps.tile([P, 2, ABLK], f32,
                                                    tag="st", name="op")
                            op = op_pair[:, cb, :]
                        else:
                            op = accps.tile([P, ABLK], f32, tag="acc",
                                            name="op")[:]
                        for h in range(QPH):
                            nc.tensor.matmul(
                                op,
                                oT_t[:, h, off * P:(off + 1) * P],
                                wo_sb[:, h, cb * ABLK:(cb + 1) * ABLK],
                                start=(h == 0), stop=(h == QPH - 1))
                        ob = osb.tile([P, ABLK], f32, tag="ob", name="ob")
                        nc.scalar.activation(ob[:], op, COPY)
                        nc.sync.dma_start(
                            out=outp[t * P:(t + 1) * P,
                                     cb * ABLK:(cb + 1) * ABLK],
                            in_=ob[:])

    _strip_pe_self_waits(nc)
    nc.finalize()
    return nc


def _strip_pe_self_waits(nc):
    """Remove PE-on-PE semaphore waits from PE matmuls (always satisfied
    by program order; frees the single sync-wait slot of self-loading
    matmul forms for real cross-engine deps)."""
    import concourse.mybir as mybir

    stripped = 0
    for bb in nc.m.functions[0].blocks:
        for inst in bb.instructions:
            si = getattr(inst, "sync_info", None)
            if si is None or not getattr(si, "on_wait", None):
                continue
            if isinstance(inst, mybir.InstMatmult):
                keep = [
                    w for w in si.on_wait
                    if not (w.sync_type == "semaphore"
                            and w.ant_name.startswith("PE"))
                ]
                stripped += len(si.on_wait) - len(keep)
                si.on_wait = keep
    return stripped


def _prep_inputs(x, freqs_cos, freqs_sin, Wq, Wk, Wv, Wo):
    """Build the 8 per-core input maps (layout + bf16 cast only)."""
    from ml_dtypes import bfloat16

    perm = np.concatenate([np.arange(0, HD, 2), np.arange(1, HD, 2)])

    cosT = np.ascontiguousarray(freqs_cos.T).astype(bfloat16)  # [64, S]
    sinT = np.ascontiguousarray(freqs_sin.T).astype(bfloat16)
    ones = np.ones((P, P), np.float32).astype(bfloat16)
    # st[sk, sq']: keep sk <= sq' (incl. diagonal)
    mask01 = np.triu(np.ones((P, P), np.float32)).astype(bfloat16)

    xTs = [np.ascontiguousarray(x[b].T).astype(bfloat16) for b in range(B)]

    w1s, w2s, w3s, wos = [], [], [], []
    for g in range(G):
        wq_g = Wq[:, g * EQ:(g + 1) * EQ].reshape(D, QPH, HD)[:, :, perm]
        wk_g = Wk[:, g * HD:(g + 1) * HD][:, perm]
        wv_g = Wv[:, g * HD:(g + 1) * HD]
        w1s.append(np.ascontiguousarray(
            np.concatenate([wk_g, wq_g[:, 0]], axis=1)).astype(bfloat16))
        w2s.append(np.ascontiguousarray(
            np.concatenate([wq_g[:, 1], wq_g[:, 2]], axis=1)).astype(bfloat16))
        w3s.append(np.ascontiguousarray(
            np.concatenate([wq_g[:, 3], wv_g], axis=1)).astype(bfloat16))
        wos.append(np.ascontiguousarray(
            Wo[g * EQ:(g + 1) * EQ, :]).astype(bfloat16))

    in_maps = []
    for c in range(NCORES):
        b, g = divmod(c, G)
        in_maps.append(
            dict(xt=xTs[b], w1=w1s[g], w2=w2s[g], w3=w3s[g], wo=wos[g],
                 cosT=cosT, sinT=sinT, ones_d=ones, mask01_d=mask01)
        )
    return in_maps


LAST_RESULTS = None


def kernel(**inputs) -> np.ndarray:
    global LAST_RESULTS
    x = np.asarray(inputs["x"], np.float32)
    in_maps = _prep_inputs(
        x,
        np.asarray(inputs["freqs_cos"], np.float32),
        np.asarray(inputs["freqs_sin"], np.float32),
        np.asarray(inputs["Wq"], np.float32),
        np.asarray(inputs["Wk"], np.float32),
        np.asarray(inputs["Wv"], np.float32),
        np.asarray(inputs["Wo"], np.float32),
    )

    if "nc" not in _CACHE:
        _CACHE["nc"] = _build_program()
    nc = _CACHE["nc"]

    from concourse import bass_utils

    res = bass_utils.run_bass_kernel_spmd(nc, in_maps, list(range(NCORES)))
    LAST_RESULTS = res

    out = np.empty((B, S, D), np.float32)
    for b in range(B):
        acc = res.results[4 * b]["outp"].astype(np.float32)
        for g in range(1, G):
            acc = acc + res.results[4 * b + g]["outp"]
        out[b] = acc
    return out


# revision 9
# speedup vs baseline: 2.1727x; 2.1727x over previous
"""Trainium2 Bass kernel for nn_Attention_49598282334528.

Dense transformer attention block: fused QKV projection + RoPE + causal
GQA attention + output projection, for
  x: [2, 2048, 2048], H=16 q heads, KV=4 kv heads, head_dim=128.

Sharding (8 NeuronCores): data-parallel over batch (2) x tensor-parallel
over kv-head groups (4).  Core c handles batch c//4, kv-group c%4 (4 q
heads + 1 kv head).  Each core computes a full-width partial of the
output projection (row-parallel Wo); the host sums the 4 partials per
batch (f32) and stacks batches.

Design (all PE operands bf16, PSUM f32; ~228.7us TimelineSim vs 348.9us
for the fp32r 3-phase baseline):
  - single software-pipelined loop over 4 seq-blocks of 512:
    proj rounds -> attention -> out-projection per block.
  - QKV projection: 6 single-output rounds (k,q0..q3,v) rotating 2 PSUM
    banks; each round is evicted to SBUF bf16 by a fast ACT/DVE copy
    pair (top/bot halves land partition-0-based) and RoPE runs all-bf16
    on DVE in 2x mode, hidden under the next round's matmuls.  q3's
    rope is deferred past the v eviction so attention exps aren't
    queued behind it on ACT.  RoPE pair-interleave is pre-permuted into
    the Wq/Wk columns host-side.
  - v is projected directly in [s, e] orientation (lhsT = x chunks),
    so no PE transposes.
  - attention processes score tiles in PAIRS sharing a 2-bank PSUM
    tile: one exp covers both halves of full pairs (amortizes the ACT
    PSUM-access overhead); causal diagonal handled by multiplying exp
    scores with a 0/1 tile on DVE.  Softmax denominators are pre-summed
    on DVE (quads of full tiles, region-wise quad of the 4 diagonal
    tiles) so ones-matmuls shrink ~4x.  Scores are issued one pair
    ahead of PV so the PE rarely waits on the exp.
  - ot/sm accumulator banks parity-swap each head so the next head's
    PV only waits on the cheap reciprocal, not the normalize mul.
  - out-projection shares the score-pair PSUM slots (2 chunks per
    2-bank tile) plus the acc slots, output partials are written bf16
    (halves the out-DMA; host reduction in f32).
  - startup DMAs chunked/ordered so the first matmul starts at ~2us.
"""

import sys

if "/opt/trn_rl_repo" not in sys.path:
    sys.path.insert(0, "/opt/trn_rl_repo")

import numpy as np

B, S, D = 2, 2048, 2048
H, KV, HD = 16, 4, 128
G = 4                # kv groups == cores per batch
QPH = H // KV        # q heads per group = 4
EQ = QPH * HD        # per-core q width = 512
NCORES = 8
P = 128
ABLK = 512           # seq block
NA = S // ABLK       # 4
ND = D // P          # 16 contraction chunks
SCALE = 1.0 / float(np.sqrt(HD))

_CACHE = {}


def _build_program():
    import concourse.bass as bass
    import concourse.tile as tile
    from concourse import bacc, mybir

    f32 = mybir.dt.float32
    bf16 = mybir.dt.bfloat16
    EXP = mybir.ActivationFunctionType.Exp
    COPY = mybir.ActivationFunctionType.Copy

    nc = bacc.Bacc("TRN2", target_bir_lowering=False, debug=False)

    xt = nc.dram_tensor("xt", [D, S], bf16, kind="ExternalInput").ap()
    w1 = nc.dram_tensor("w1", [D, 2 * P], bf16, kind="ExternalInput").ap()
    w2 = nc.dram_tensor("w2", [D, 2 * P], bf16, kind="ExternalInput").ap()
    w3 = nc.dram_tensor("w3", [D, 2 * P], bf16, kind="ExternalInput").ap()
    wo = nc.dram_tensor("wo", [EQ, D], bf16, kind="ExternalInput").ap()
    cosT = nc.dram_tensor("cosT", [HD // 2, S], bf16, kind="ExternalInput").ap()
    sinT = nc.dram_tensor("sinT", [HD // 2, S], bf16, kind="ExternalInput").ap()
    ones_d = nc.dram_tensor("ones_d", [P, P], bf16, kind="ExternalInput").ap()
    mask01_d = nc.dram_tensor("mask01_d", [P, P], bf16, kind="ExternalInput").ap()
    outp = nc.dram_tensor("outp", [S, D], bf16, kind="ExternalOutput").ap()

    xt_r = xt.rearrange("(o p) s -> p o s", p=P)     # [128, 16, 2048]
    w1_r = w1.rearrange("(o p) e -> p o e", p=P)     # [128, 16, 256]
    w2_r = w2.rearrange("(o p) e -> p o e", p=P)
    w3_r = w3.rearrange("(o p) e -> p o e", p=P)
    wo_r = wo.rearrange("(h p) d -> p h d", p=P)     # [128, 4, 2048]

    HH = HD // 2

    with tile.TileContext(nc) as tc:
        import contextlib

        with contextlib.ExitStack() as stack:
            const = stack.enter_context(tc.tile_pool(name="const", bufs=1))
            wpool = stack.enter_context(tc.tile_pool(name="wpool", bufs=1))
            xpool = stack.enter_context(tc.tile_pool(name="xpool", bufs=1))
            qkv = stack.enter_context(tc.tile_pool(name="qkv", bufs=1))
            oTp = stack.enter_context(tc.tile_pool(name="oTp", bufs=2))
            ropet = stack.enter_context(tc.tile_pool(name="ropet", bufs=4))
            stsb = stack.enter_context(tc.tile_pool(name="stsb", bufs=6))
            rcp = stack.enter_context(tc.tile_pool(name="rcp", bufs=2))
            osb = stack.enter_context(tc.tile_pool(name="osb", bufs=4))
            # PSUM budget (8 banks): proj rounds 2 + score-pair/outproj
            # 2-bank tiles x2 + attention-out/denominator 2 (parity-swapped
            # so h-transitions wait on the cheap reciprocal, not the mul).
            projps = stack.enter_context(
                tc.tile_pool(name="projps", bufs=2, space="PSUM"))
            stps = stack.enter_context(
                tc.tile_pool(name="stps", bufs=2, space="PSUM"))
            accps = stack.enter_context(
                tc.tile_pool(name="accps", bufs=2, space="PSUM"))

            # ---- SBUF persistents ----
            cos_sb = const.tile([HH, S], bf16)
            sin_sb = const.tile([HH, S], bf16)
            ones_sb = const.tile([P, P], bf16)
            mask01_sb = const.tile([P, P], bf16)

            w1_sb = wpool.tile([P, ND, 2 * P], bf16)
            w2_sb = wpool.tile([P, ND, 2 * P], bf16)
            w3_sb = wpool.tile([P, ND, 2 * P], bf16)
            wo_sb = wpool.tile([P, QPH, D], bf16)

            xb_sb = [xpool.tile([P, ND, ABLK], bf16, name=f"xb{b}")
                     for b in range(NA)]

            qT = [qkv.tile([P, QPH, ABLK], bf16, name=f"qT{b}")
                  for b in range(NA)]
            kT = [qkv.tile([P, ABLK], bf16, name=f"kT{b}") for b in range(NA)]
            vS = [qkv.tile([P, ABLK // P, HD], bf16, name=f"v{b}")
                  for b in range(NA)]

            # ---- DMA schedule: startup chunked so PE starts ASAP.
            # Interleave w1 / x-block-0 chunks, then consts (needed by the
            # first rope), then w2/w3 just in time for their rounds.
            NCH = 4
            DCH = ND // NCH  # 4 di per chunk
            def wchunk(sb, r, c):
                nc.sync.dma_start(out=sb[:, c * DCH:(c + 1) * DCH, :],
                                  in_=r[:, c * DCH:(c + 1) * DCH, :])
            def xchunk(b, c):
                nc.sync.dma_start(
                    out=xb_sb[b][:, c * DCH:(c + 1) * DCH, :],
                    in_=xt_r[:, c * DCH:(c + 1) * DCH,
                             b * ABLK:(b + 1) * ABLK])
            wchunk(w1_sb, w1_r, 0)
            xchunk(0, 0)
            nc.sync.dma_start(out=cos_sb[:], in_=cosT[:])
            nc.sync.dma_start(out=sin_sb[:], in_=sinT[:])
            wchunk(w1_sb, w1_r, 1)
            xchunk(0, 1)
            wchunk(w1_sb, w1_r, 2)
            xchunk(0, 2)
            wchunk(w1_sb, w1_r, 3)
            xchunk(0, 3)
            wchunk(w2_sb, w2_r, 0)
            wchunk(w2_sb, w2_r, 1)
            wchunk(w2_sb, w2_r, 2)
            wchunk(w2_sb, w2_r, 3)
            nc.sync.dma_start(out=ones_sb[:], in_=ones_d[:])
            nc.sync.dma_start(out=mask01_sb[:], in_=mask01_d[:])
            nc.sync.dma_start(out=w3_sb[:, 0:8, :], in_=w3_r[:, 0:8, :])
            nc.sync.dma_start(out=w3_sb[:, 8:16, :], in_=w3_r[:, 8:16, :])
            nc.sync.dma_start(out=xb_sb[1][:], in_=xt_r[:, :, ABLK:2 * ABLK])
            nc.sync.dma_start(out=wo_sb[:], in_=wo_r[:])
            nc.sync.dma_start(out=xb_sb[2][:], in_=xt_r[:, :, 2 * ABLK:3 * ABLK])
            nc.sync.dma_start(out=xb_sb[3][:], in_=xt_r[:, :, 3 * ABLK:4 * ABLK])

            def rope(top, bot, dst, s0):
                """top/bot: SBUF bf16 [64, ABLK] partition-0-based (even /
                odd dims); dst: SBUF bf16 [128, ABLK] slice.  All-SBUF
                bf16 operands at the same start partition -> legal
                TensorTensor + DVE 2x mode."""
                ct = cos_sb[:, s0:s0 + ABLK]
                st_ = sin_sb[:, s0:s0 + ABLK]
                t1 = ropet.tile([HH, ABLK], bf16, tag="t1", name="rt1")
                t2 = ropet.tile([HH, ABLK], bf16, tag="t2", name="rt2")
                nc.vector.tensor_mul(t1[:], top, ct)
                nc.vector.tensor_mul(t2[:], bot, st_)
                nc.vector.tensor_sub(dst[0:HH, :], t1[:], t2[:])
                t3 = ropet.tile([HH, ABLK], bf16, tag="t1", name="rt1")
                t4 = ropet.tile([HH, ABLK], bf16, tag="t2", name="rt2")
                nc.vector.tensor_mul(t3[:], top, st_)
                nc.vector.tensor_mul(t4[:], bot, ct)
                nc.vector.tensor_add(dst[HH:P, :], t3[:], t4[:])

            for b in range(NA):
                s0 = b * ABLK
                xb = xb_sb[b]

                # ---- projection: 6 single-output rounds (k,q0..q3,v) so
                # RoPE of round i (DVE) hides under round i+1's matmuls
                # while only 2 PSUM banks rotate.
                rounds = [
                    (w1_sb, 0, kT[b][:]),
                    (w1_sb, 1, qT[b][:, 0, :]),
                    (w2_sb, 0, qT[b][:, 1, :]),
                    (w2_sb, 1, qT[b][:, 2, :]),
                    (w3_sb, 0, qT[b][:, 3, :]),
                ]
                q3_pending = None
                for ri, (wsb, col, dst) in enumerate(rounds):
                    rp = projps.tile([P, ABLK], f32, tag="proj", name="rp")
                    for di in range(ND):
                        nc.tensor.matmul(
                            rp[:], wsb[:, di, col * P:(col + 1) * P],
                            xb[:, di, :], start=di == 0, stop=di == ND - 1)
                    if ri == len(rounds) - 1:
                        # q3 is only needed at head 3: defer its evict+rope
                        # (all-DVE) until after the v eviction so the first
                        # attention exps aren't queued behind it on ACT.
                        q3_pending = (rp, dst)
                        continue
                    # fast evicts free the PSUM slot; each half lands in
                    # its own partition-0-based tile so the rope TensorTensor
                    # ops are legal (the PSUM source of the evict is exempt
                    # from the same-start-partition rule).
                    rtop = rpeh.tile([HH, ABLK], bf16, tag="rtop",
                                     name="rtop")
                    rbot = rpeh.tile([HH, ABLK], bf16, tag="rbot",
                                     name="rbot")
                    nc.scalar.activation(rtop[:], rp[0:HH, :], COPY)
                    nc.vector.tensor_copy(rbot[:], rp[HH:P, :])
                    rope(rtop[:], rbot[:], dst, s0)
                # v directly in [s, e] orientation (lhsT = x chunk)
                # accumulation groups must be sequential within a PSUM bank:
                # j outer (one group per s-tile), di inner.
                vt = projps.tile([P, ABLK // P, HD], f32, tag="proj",
                                 name="vt")
                for j in range(ABLK // P):
                    for di in range(ND):
                        nc.tensor.matmul(
                            vt[:, j, :], xb[:, di, j * P:(j + 1) * P],
                            w3_sb[:, di, P:2 * P],
                            start=di == 0, stop=di == ND - 1)
                nc.scalar.activation(vS[b][:], vt[:], COPY)
                rp, dst = q3_pending
                rtop = rpeh.tile([HH, ABLK], bf16, tag="rtop", name="rtop")
                rbot = rpeh.tile([HH, ABLK], bf16, tag="rbot", name="rbot")
                nc.vector.tensor_copy(rtop[:], rp[0:HH, :])
                nc.vector.tensor_copy(rbot[:], rp[HH:P, :])
                rope(rtop[:], rbot[:], dst, s0)

                # ---- attention for block-row b ----
                # Score tiles processed in PAIRS sharing a 2-bank PSUM
                # tile: full pairs get ONE exp over both halves; the
                # denominators of each 4 full tiles are pre-summed on DVE
                # (quad) so one ones-matmul covers them.  Pipelined one
                # pair ahead.  ot/sm slots parity-swap each head so the
                # next head's PV waits only on the reciprocal.
                n_sk = (s0 + ABLK) // P
                n_pair = n_sk // 2
                n_full = 2 * b   # full pairs per head (then 2 diag pairs)
                oT_t = oTp.tile([P, QPH, ABLK], bf16, tag="oT", name="oT")

                def kslice(ki):
                    return kT[ki // (ABLK // P)][
                        :, (ki % (ABLK // P)) * P:(ki % (ABLK // P) + 1) * P]

                def issue_pair(h, p):
                    ki0 = 2 * p
                    stp = stps.tile([P, 2, ABLK], f32, tag="st", name="stp")
                    stt = stsb.tile([P, 2, ABLK], bf16, tag="stsb",
                                    name="stt")
                    for half in range(2):
                        ki = ki0 + half
                        lead = max(ki * P - s0, 0)
                        nc.tensor.matmul(
                            stp[:, half, lead:], kslice(ki),
                            qT[b][:, h, lead:], start=True, stop=True)
                    if p < n_full:
                        nc.scalar.activation(stt[:, :, :], stp[:, :, :],
                                             EXP, scale=SCALE)
                    else:
                        for half in range(2):
                            ki = ki0 + half
                            lead = ki * P - s0
                            nc.scalar.activation(
                                stt[:, half, lead:], stp[:, half, lead:],
                                EXP, scale=SCALE)
                            nc.vector.tensor_mul(
                                stt[:, half, lead:lead + P],
                                stt[:, half, lead:lead + P], mask01_sb[:])
                    return stt

                iters = [(h, p) for h in range(QPH) for p in range(n_pair)]
                pending = issue_pair(*iters[0])
                ot = sm = qsum = None
                for idx, (h, p) in enumerate(iters):
                    stt = pending
                    if idx + 1 < len(iters):
                        pending = issue_pair(*iters[idx + 1])
                    if p == 0:
                        if h % 2 == 0:
                            ot = accps.tile([P, ABLK], f32, tag="acc",
                                            name="ot")
                            sm = accps.tile([P, ABLK], f32, tag="acc",
                                            name="sm")
                        else:
                            sm = accps.tile([P, ABLK], f32, tag="acc",
                                            name="sm")
                            ot = accps.tile([P, ABLK], f32, tag="acc",
                                            name="ot")
                    for half in range(2):
                        ki = 2 * p + half
                        lead = max(ki * P - s0, 0)
                        nc.tensor.matmul(
                            ot[:, lead:],
                            vS[ki // (ABLK // P)][:, ki % (ABLK // P), :],
                            stt[:, half, lead:],
                            start=ki == 0, stop=ki == n_sk - 1)
                    if p < n_full:
                        # denominator: accumulate 2 pairs (4 tiles) on DVE,
                        # then a single ones-matmul per quad.
                        if p % 2 == 0:
                            qsum = stsb.tile([P, ABLK], bf16, tag="qsum",
                                             name="qsum")
                            nc.vector.tensor_add(qsum[:], stt[:, 0, :],
                                                 stt[:, 1, :])
                        else:
                            nc.vector.tensor_add(qsum[:], qsum[:],
                                                 stt[:, 0, :])
                            nc.vector.tensor_add(qsum[:], qsum[:],
                                                 stt[:, 1, :])
                            nc.tensor.matmul(
                                sm[:], ones_sb[:], qsum[:],
                                start=p == 1, stop=False)
                    elif p == n_full:
                        # first diag pair: start region-wise accumulation
                        # of the 4 diagonal tiles on DVE.
                        qsum = stsb.tile([P, ABLK], bf16, tag="qsum",
                                         name="qsum")
                        nc.vector.tensor_copy(qsum[:, 0:P], stt[:, 0, 0:P])
                        nc.vector.tensor_add(qsum[:, P:], stt[:, 0, P:],
                                             stt[:, 1, P:])
                    else:
                        # second diag pair: finish the sum, single
                        # ones-matmul for all four diagonal tiles.
                        nc.vector.tensor_add(qsum[:, 2 * P:], qsum[:, 2 * P:],
                                             stt[:, 0, 2 * P:])
                        nc.vector.tensor_add(qsum[:, 3 * P:], qsum[:, 3 * P:],
                                             stt[:, 1, 3 * P:])
                        nc.tensor.matmul(
                            sm[:], ones_sb[:], qsum[:],
                            start=b == 0, stop=True)
                    if p == n_pair - 1:
                        rc = rcp.tile([P, ABLK], f32, tag="rc", name="rc")
                        nc.vector.reciprocal(rc[:], sm[:])
                        nc.vector.tensor_mul(oT_t[:, h, :], ot[:], rc[:])

                # ---- output projection for block-row b (partial) ----
                # shares the score-pair PSUM slots; two column-chunks per
                # 2-bank tile.
                op_pair = None
                for off in range(ABLK // P):
                    t = b * (ABLK // P) + off
                    for cb in range(D // ABLK):
                        if cb < 2:
                            if cb == 0:
                                op_pair = stps.tile([P, 2, ABLK], f32,
                                                    tag="st", name="op")
                            op = op_pair[:, cb, :]
                        else:
                            op = accps.tile([P, ABLK], f32, tag="acc",
                                            name="op")[:]
                        for h in range(QPH):
                            nc.tensor.matmul(
                                op,
                                oT_t[:, h, off * P:(off + 1) * P],
                                wo_sb[:, h, cb * ABLK:(cb + 1) * ABLK],
                                start=(h == 0), stop=(h == QPH - 1))
                        ob = osb.tile([P, ABLK], bf16, tag="ob", name="ob")
                        if b == NA - 1 and off >= 2 and cb % 2:
                            nc.vector.tensor_copy(ob[:], op)
                        else:
                            nc.scalar.activation(ob[:], op, COPY)
                        nc.sync.dma_start(
                            out=outp[t * P:(t + 1) * P,
                                     cb * ABLK:(cb + 1) * ABLK],
                            in_=ob[:])

    _strip_pe_self_waits(nc)
    nc.finalize()
    return nc


def _strip_pe_self_waits(nc):
    """Remove PE-on-PE semaphore waits from PE matmuls (always satisfied
    by program order; frees the single sync-wait slot of self-loading
    matmul forms for real cross-engine deps)."""
    import concourse.mybir as mybir

    stripped = 0
    for bb in nc.m.functions[0].blocks:
        for inst in bb.instructions:
            si = getattr(inst, "sync_info", None)
            if si is None or not getattr(si, "on_wait", None):
                continue
            if isinstance(inst, mybir.InstMatmult):
                keep = [
                    w for w in si.on_wait
                    if not (w.sync_type == "semaphore"
                            and w.ant_name.startswith("PE"))
                ]
                stripped += len(si.on_wait) - len(keep)
                si.on_wait = keep
    return stripped


def _prep_inputs(x, freqs_cos, freqs_sin, Wq, Wk, Wv, Wo):
    """Build the 8 per-core input maps (layout + bf16 cast only)."""
    from ml_dtypes import bfloat16

    perm = np.concatenate([np.arange(0, HD, 2), np.arange(1, HD, 2)])

    cosT = np.ascontiguousarray(freqs_cos.T).astype(bfloat16)  # [64, S]
    sinT = np.ascontiguousarray(freqs_sin.T).astype(bfloat16)
    ones = np.ones((P, P), np.float32).astype(bfloat16)
    # st[sk, sq']: keep sk <= sq' (incl. diagonal)
    mask01 = np.triu(np.ones((P, P), np.float32)).astype(bfloat16)

    xTs = [np.ascontiguousarray(x[b].T).astype(bfloat16) for b in range(B)]

    w1s, w2s, w3s, wos = [], [], [], []
    for g in range(G):
        wq_g = Wq[:, g * EQ:(g + 1) * EQ].reshape(D, QPH, HD)[:, :, perm]
        wk_g = Wk[:, g * HD:(g + 1) * HD][:, perm]
        wv_g = Wv[:, g * HD:(g + 1) * HD]
        w1s.append(np.ascontiguousarray(
            np.concatenate([wk_g, wq_g[:, 0]], axis=1)).astype(bfloat16))
        w2s.append(np.ascontiguousarray(
            np.concatenate([wq_g[:, 1], wq_g[:, 2]], axis=1)).astype(bfloat16))
        w3s.append(np.ascontiguousarray(
            np.concatenate([wq_g[:, 3], wv_g], axis=1)).astype(bfloat16))
        wos.append(np.ascontiguousarray(
            Wo[g * EQ:(g + 1) * EQ, :]).astype(bfloat16))

    in_maps = []
    for c in range(NCORES):
        b, g = divmod(c, G)
        in_maps.append(
            dict(xt=xTs[b], w1=w1s[g], w2=w2s[g], w3=w3s[g], wo=wos[g],
                 cosT=cosT, sinT=sinT, ones_d=ones, mask01_d=mask01)
        )
    return in_maps


LAST_RESULTS = None


def kernel(**inputs) -> np.ndarray:
    global LAST_RESULTS
    x = np.asarray(inputs["x"], np.float32)
    in_maps = _prep_inputs(
        x,
        np.asarray(inputs["freqs_cos"], np.float32),
        np.asarray(inputs["freqs_sin"], np.float32),
        np.asarray(inputs["Wq"], np.float32),
        np.asarray(inputs["Wk"], np.float32),
        np.asarray(inputs["Wv"], np.float32),
        np.asarray(inputs["Wo"], np.float32),
    )

    if "nc" not in _CACHE:
        _CACHE["nc"] = _build_program()
    nc = _CACHE["nc"]

    from concourse import bass_utils

    res = bass_utils.run_bass_kernel_spmd(nc, in_maps, list(range(NCORES)))
    LAST_RESULTS = res

    out = np.empty((B, S, D), np.float32)
    for b in range(B):
        acc = res.results[4 * b]["outp"].astype(np.float32)
        for g in range(1, G):
            acc = acc + res.results[4 * b + g]["outp"]
        out[b] = acc
    return out
